# revision 29
# baseline (speedup 1.0000x reference)
"""Trainium2 Bass kernel for DecomposingAttnProcessor (pooled component softmax
cross-attention), sharded over 8 NeuronCores along the batch-component axis.

Math (per batch-component bc = c*B + b):
    q = x @ Wq ; k = enc @ Wk ; v = enc @ Wv           (per-head, dh = 64)
    scores = (q k^T) * dh^-0.5                          [H, S, E]
    pooled = mean_E scores ; wp = softmax_c(pooled)     (couples components)
    w = softmax_E(scores) * wp
    out = (w v) @ Wo + bo + x

V8 design: core i owns bc = i (full S = 4096).  The only cross-component
coupling is sum_c exp(pooled) -- a [16, 512] f32 AllReduce per 512-row
s-chunk across the 4 cores sharing the same b (groups {0,2,4,6} / {1,3,5,7}),
issued right after the pooled matmuls so it completes behind the chunk's
~40us attention head loop.  Everything else is local.

The host supplies x and enc pre-transposed (xT, encT) plus the selector
constant, so the kernel contains NO PE transposes -- which lets the walrus
LDWEIGHTS optimization compile, cutting per-matmul stationary-load overhead.

  - dh^-0.5 folded into kT at the encoder stage.
  - scoresT[e, s] per head; AV head-pairs share a PSUM bank at bases 0/64 ->
    one [128, 512] eviction per pair.
  - softmax denominators emitted as a stacked [16, 512] PSUM block (own
    bank, base 0) via zero-padded ones-column matmuls (lhsT = Z[:, h:16],
    col 15 ones): den of head h lands on partition 15-h.
  - pooled rows stack at base 0 of a transient bank; exp+AllReduce up front.
  - coef = exp(pooled/E)/allsum/den on [16, 512] tiles; broadcast across
    partitions via a PE selector matmul into PSUM; DVE multiplies ao in
    place with in2 = PSUM (no DRAM bounce).
  - D/E (coef-apply + O-projection) of chunk sc-1 interleave into chunk sc's
    A/B as stall fillers.
"""

import os
import sys
from contextlib import ExitStack

sys.path.insert(0, "/opt/trn_rl_repo")

import numpy as np

import concourse.bass as bass  # noqa: E402
import concourse.bass_utils as _bass_utils  # noqa: E402
from concourse import bacc, mybir  # noqa: E402
from concourse.bass_utils import run_bass_kernel_spmd  # noqa: E402
from concourse.tile import TileContext  # noqa: E402

NO_CC = os.environ.get("NO_CC", "0") == "1"
LDW_OPT = os.environ.get("LDW_OPT", "0") == "1"

_orig_run_command = _bass_utils.run_command


def _run_command_ldwopt(argv, **kwargs):
    argv = ["--enable-ldw-opt=true" if a == "--enable-ldw-opt=false" else a for a in argv]
    return _orig_run_command(argv, **kwargs)


if LDW_OPT:
    _bass_utils.run_command = _run_command_ldwopt

# Problem dims (hardcoded per spec)
BC, S, D, E, H, C = 8, 4096, 1024, 160, 16, 4
B = BC // C  # 2
DH = D // H  # 64
SCALE = DH**-0.5  # 0.125
N_CORES = 8
E0, E1 = 128, E - 128  # encoder-token chunks (128 + 32)
ND = D // 128  # 8 chunks of the hidden dim
SL = 512  # s-chunk rows per iteration
NSC = S // SL  # 8 chunks
REPLICA_GROUPS = [[0, 2, 4, 6], [1, 3, 5, 7]]  # cores sharing the same b

F32 = mybir.dt.float32
BF16 = mybir.dt.bfloat16
EXP = mybir.ActivationFunctionType.Exp
COPY = mybir.ActivationFunctionType.Copy


def build_body(ctx, tc, d):
    nc = tc.nc
    ctx.enter_context(
        nc.allow_low_precision(reason="bf16 stats are within the 2e-2 rel-err budget")
    )
    P = 128

    pools = {}

    def pool(name, bufs, space="SBUF"):
        if name not in pools:
            pools[name] = ctx.enter_context(tc.tile_pool(name=name, bufs=bufs, space=space))
        return pools[name]

    const = pool("const", 1)
    wres = pool("wres", 1)    # Wq / Wo resident bf16
    kv_p = pool("kv", 1)      # kt / v0 / v1 / ksb / enct
    xt_p = pool("xt", 2)
    qt_p = pool("qt", 2)
    wa_p = pool("wa", 2)
    wb_p = pool("wb", 2)
    ao_p = pool("ao", 3)      # also hosts Wk/Wv during the encoder phase
    st_p = pool("st", 2)
    wstage = pool("wstage", 2)
    xr_p = pool("xr", 2)
    oh_p = pool("oh", 2)
    dram = pool("dram", 1, space="DRAM")

    # PSUM: 8 banks
    psA = pool("psA", 2, space="PSUM")    # E0 scores / kT-proj
    psEAV = pool("psEAV", 3, space="PSUM")  # E1 pairs + AV pairs / v-proj
    psO = pool("psO", 2, space="PSUM")    # Q-proj / pooled / cb / O-proj
    psDN = pool("psDN", 1, space="PSUM")  # denominator stack rows 0:16

    # ---- constants ----
    # Z: ones at col 15 only; lhsT = Z[rows, h:16] puts a ones-column at out
    # partition 15-h with zeros accumulated above it (den stack).
    zden = const.tile([P, 16], BF16, tag="zden")
    nc.vector.memset(zden, 0.0)
    nc.vector.memset(zden[:, 15:16], 1.0)
    # sel[j] = selc[:, 128j:128j+128]: cb[p, s] = coef[15 - (2j + (p>=64)), s]
    selc = const.tile([16, ND * P], BF16, tag="selc")
    nc.gpsimd.dma_start(out=selc, in_=d["selc"])

    # ---- encoder inputs first (PE starts on them), then weights ----
    enct = kv_p.tile([P, ND * E], BF16, tag="enct")
    nc.gpsimd.dma_start(
        out=enct, in_=d["encT"].rearrange("(n p) e -> p n e", p=P)
    )
    wq = wres.tile([P, ND * D], BF16, tag="wq")
    wo = wres.tile([P, ND * D], BF16, tag="wo")
    wk_lo = ao_p.tile([P, ND * SL], BF16, tag="ao", name="wk_lo", bufs=4)
    wk_hi = ao_p.tile([P, ND * SL], BF16, tag="ao", name="wk_hi", bufs=4)
    wv_lo = ao_p.tile([P, ND * SL], BF16, tag="ao", name="wv_lo", bufs=4)
    wv_hi = ao_p.tile([P, ND * SL], BF16, tag="ao", name="wv_hi", bufs=4)
    # f32 staged via the HWDGE (sync) queue + ACT cast: the gpsimd software
    # cast queue would serialize ~24MB in front of everything else.
    for nm, dsts in (("Wk", (wk_lo, wk_hi)), ("Wv", (wv_lo, wv_hi)),
                     ("Wq", (wq[:, 0 : 4 * D], wq[:, 4 * D : 8 * D])),
                     ("Wo", (wo[:, 0 : 4 * D], wo[:, 4 * D : 8 * D]))):
        for hf in range(2):
            stg = wstage.tile([P, 4 * D], F32, tag="wstg", name=f"wstg_{nm}{hf}")
            nc.sync.dma_start(
                out=stg, in_=d[nm].rearrange("(n p) d -> p n d", p=P)[:, 4 * hf : 4 * hf + 4, :]
            )
            nc.scalar.activation(dsts[hf], stg, COPY)

    def wslice(lo, hi, i, c0, c1):
        t = lo if i < 4 else hi
        return t[:, D * (i % 4) + c0 : D * (i % 4) + c1]

    # ---- encoder phase (this core's bc only) ----
    kt = kv_p.tile([P, ND * E], BF16, tag="kt")
    ksb = {}
    for j in range(ND):
        ps = psA.tile([P, 512], F32, tag="ps", name="psk")
        for i in range(ND):
            nc.tensor.matmul(
                ps[:, 0:E],
                lhsT=wslice(wk_lo, wk_hi, i, 128 * j, 128 * (j + 1)),
                rhs=enct[:, E * i : E * (i + 1)],
                start=(i == 0),
                stop=(i == ND - 1),
            )
        ksl = kt[:, E * j : E * (j + 1)]
        nc.scalar.activation(ksl, ps[:, 0:E], COPY, scale=SCALE)
        kb = kv_p.tile([P, 16], BF16, tag=f"ksb{j}", name=f"ksb{j}")
        nc.gpsimd.memset(kb, 0.0)
        # head 2j ksum -> col 15-2j (rows 0:64); head 2j+1 -> col 14-2j
        nc.vector.tensor_reduce(
            kb[0:64, 15 - 2 * j : 16 - 2 * j], ksl[0:64, :],
            axis=mybir.AxisListType.X, op=mybir.AluOpType.add,
        )
        nc.vector.tensor_reduce(
            kb[64:128, 14 - 2 * j : 15 - 2 * j], ksl[64:128, :],
            axis=mybir.AxisListType.X, op=mybir.AluOpType.add,
        )
        ksb[j] = kb
    v0 = kv_p.tile([P, D], BF16, tag="v0")
    v1 = kv_p.tile([P, D], BF16, tag="v1")
    for half in range(2):
        cols = slice(512 * half, 512 * (half + 1))
        ps0 = psEAV.tile([P, 512], F32, tag="ps", name="psv0")
        ps1 = psEAV.tile([P, 512], F32, tag="ps", name="psv1")
        for i in range(ND):
            nc.tensor.matmul(
                ps0, lhsT=enct[:, E * i : E * i + E0],
                rhs=wslice(wv_lo, wv_hi, i, 512 * half, 512 * (half + 1)),
                start=(i == 0), stop=(i == ND - 1),
            )
        for i in range(ND):
            nc.tensor.matmul(
                ps1[0:E1, :], lhsT=enct[:, E * i + E0 : E * i + E],
                rhs=wslice(wv_lo, wv_hi, i, 512 * half, 512 * (half + 1)),
                start=(i == 0), stop=(i == ND - 1),
            )
        nc.scalar.activation(v0[:, cols], ps0, COPY)
        # replicate v1 rows at partition bases 0 and 64
        nc.scalar.activation(v1[0:E1, cols], ps1[0:E1, :], COPY)
        nc.vector.tensor_copy(v1[64 : 64 + E1, cols], ps1[0:E1, :])

    # ---- main loop over s-chunks ----
    xts = {}
    state = {}

    def prefetch_xt(sc):
        if sc in xts:
            return
        xt = xt_p.tile([P, ND * SL], BF16, tag="xt", name=f"xt{sc}")
        nc.gpsimd.dma_start(
            out=xt,
            in_=d["xT"].rearrange("(n p) s -> p n s", p=P)[:, :, SL * sc : SL * (sc + 1)],
        )
        xts[sc] = xt

    def emit_A(sc, xt, filler=()):
        """Q-projection for one (prefetched, pre-transposed) s-chunk."""
        filler = list(filler)[:2]
        qt = qt_p.tile([P, ND * SL], BF16, tag="qt")
        for j in range(ND):
            if j in (2, 5) and filler:
                filler.pop(0)()
            ps = psO.tile([P, 512], F32, tag="ps", name="psq")
            for i in range(ND):
                nc.tensor.matmul(
                    ps,
                    lhsT=wq[:, D * i + 128 * j : D * i + 128 * (j + 1)],
                    rhs=xt[:, SL * i : SL * (i + 1)],
                    start=(i == 0),
                    stop=(i == ND - 1),
                )
            nc.scalar.activation(qt[:, SL * j : SL * (j + 1)], ps, COPY)
        for g in filler:
            g()
        return qt

    def emit_B(sc, qt, ao, filler):
        """Scores + exp + AV + pooled + den + coef for one chunk.

        Software-pipelined: scores(j+1) is emitted before av_den(j) so the
        exp of pair j completes behind pair j+1's score matmuls.  `filler`
        callables (prev chunk's D/E) plug the remaining gaps.
        """
        if sc + 1 < NSC:
            prefetch_xt(sc + 1)
        pl = psO.tile([P, 512], F32, tag="ps", name=f"pl{sc}")
        for j in range(ND):
            nc.tensor.matmul(
                pl[0:16, :],
                lhsT=ksb[j],
                rhs=qt[:, SL * j : SL * (j + 1)],
                start=(j == 0),
                stop=(j == ND - 1),
                skip_group_check=True,
            )
        # exp(pooled/E); the AllReduce covers chunk PAIRS and is issued at
        # the odd chunk's B start -- ep of both halves exists by then and the
        # collective completes behind this chunk's head loop.
        ep = st_p.tile([16, SL], F32, tag="ep", name=f"ep{sc}", bufs=4)
        nc.scalar.activation(ep, pl[0:16, :], EXP, scale=1.0 / E)
        k = sc // 2
        if sc % 2 == 0:
            ep2_d = dram.tile([16, 2 * SL], F32, tag="ep2_d", name=f"ep2_d{k}", bufs=2)
            es2_d = dram.tile([16, 2 * SL], F32, tag="es2_d", name=f"es2_d{k}", bufs=2)
            state["ep2_d"], state["es2_d"] = ep2_d, es2_d
        else:
            ep2_d, es2_d = state["ep2_d"], state["es2_d"]
        nc.sync.dma_start(out=ep2_d[:, SL * (sc % 2) : SL * (sc % 2 + 1)], in_=ep)
        rs2 = None
        if sc % 2 == 1:
            if NO_CC:
                nc.sync.dma_start(out=es2_d, in_=ep2_d)
            else:
                nc.gpsimd.collective_compute(
                    "AllReduce",
                    mybir.AluOpType.add,
                    replica_groups=REPLICA_GROUPS,
                    ins=[ep2_d[:, :]],
                    outs=[es2_d[:, :]],
                )
            es2 = st_p.tile([16, 2 * SL], F32, tag="es2", name=f"es2_{k}")
            nc.sync.dma_start(out=es2, in_=es2_d)
            rs2 = st_p.tile([16, 2 * SL], BF16, tag="rs2", name=f"rs2_{k}")
            nc.vector.reciprocal(rs2, es2)

        dn = psDN.tile([P, 512], F32, tag="ps", name=f"dn{sc}")
        fi = 0

        def scores(j):
            psb = psEAV.tile([P, 512], F32, tag="ps", name="psb")
            was = []
            for hp in range(2):
                hr = 64 * hp
                qsl = qt[hr : hr + 64, SL * j : SL * (j + 1)]
                ps_a = psA.tile([P, 512], F32, tag="ps", name="ps_a")
                nc.tensor.matmul(
                    ps_a, lhsT=kt[hr : hr + 64, E * j : E * j + E0], rhs=qsl,
                    start=True, stop=True,
                )
                nc.tensor.matmul(
                    psb[64 * hp : 64 * hp + E1, :],
                    lhsT=kt[hr : hr + 64, E * j + E0 : E * j + E],
                    rhs=qsl,
                    start=True, stop=True, skip_group_check=True,
                )
                wa = wa_p.tile([P, SL], BF16, tag=f"wa{hp}", name=f"wa{hp}")
                nc.scalar.activation(wa, ps_a, EXP)
                was.append(wa)
            wb = wb_p.tile([P, SL], BF16, tag="wb", name="wb")
            nc.scalar.activation(wb[0 : 64 + E1, :], psb[0 : 64 + E1, :], EXP)
            return was, wb

        def av_den(j, was, wb):
            ps_av = psEAV.tile([P, 512], F32, tag="ps", name="ps_av")
            for hp in range(2):
                h = 2 * j + hp
                hr = 64 * hp
                wa = was[hp]
                wbs = wb[64 * hp : 64 * hp + E1, :]
                vsl = slice(64 * h, 64 * (h + 1))
                nc.tensor.matmul(
                    ps_av[hr : hr + 64, :], lhsT=v0[:, vsl], rhs=wa,
                    start=True, stop=False, skip_group_check=True,
                )
                nc.tensor.matmul(
                    ps_av[hr : hr + 64, :],
                    lhsT=v1[64 * hp : 64 * hp + E1, vsl],
                    rhs=wbs,
                    start=False, stop=True, skip_group_check=True,
                )
                # denominator stack: den_h -> partition 15-h (base 0)
                nc.tensor.matmul(
                    dn[0 : 16 - h, :],
                    lhsT=zden[:, h:16], rhs=wa,
                    start=(h == 0), stop=False,
                    skip_group_check=True,
                )
                nc.tensor.matmul(
                    dn[0 : 16 - h, :],
                    lhsT=zden[64 * hp : 64 * hp + E1, h:16],
                    rhs=wbs,
                    start=False, stop=(h == H - 1),
                    skip_group_check=True,
                )
            nc.vector.tensor_copy(ao[:, SL * j : SL * (j + 1)], ps_av)

        prev = None
        for j in range(ND):
            cur = scores(j)
            if j in (1, 3, 5) and fi < len(filler):
                filler[fi]()
                fi += 1
            if prev is not None:
                av_den(j - 1, *prev)
            prev = cur
        if fi < len(filler):
            filler[fi]()
            fi += 1
        av_den(ND - 1, *prev)

        # den copied out fast so the dn bank frees before the slow DVE
        # reciprocal runs.
        dencp = st_p.tile([16, SL], F32, tag="dencp", name=f"dencp{sc}")
        nc.scalar.activation(dencp, dn[0:16, :], COPY)
        rd = st_p.tile([16, SL], BF16, tag="rd", name=f"rd{sc}", bufs=4)
        nc.vector.reciprocal(rd, dencp)
        cfs = []
        if sc % 2 == 1:
            for half, (ep_h, rd_h) in enumerate(((state["ep_prev"], state["rd_prev"]), (ep, rd))):
                cf = st_p.tile([16, SL], BF16, tag="cf", name=f"cf{sc}_{half}", bufs=4)
                nc.vector.tensor_mul(rd_h, rd_h, rs2[:, SL * half : SL * (half + 1)])
                nc.vector.tensor_mul(cf, ep_h, rd_h)
                cfs.append(cf)
        else:
            state["ep_prev"], state["rd_prev"] = ep, rd
        return cfs, filler[fi:]

    def de_groups(sc, ao, cf):
        """Previous chunk's D (coef apply) + E (O-proj/store) as 8 groups."""
        groups = []

        def dgroup(j0):
            def go():
                for j in (j0, j0 + 1):
                    cb = psO.tile([P, 512], F32, tag="ps", name="cb")
                    nc.tensor.matmul(
                        cb, lhsT=selc[:, 128 * j : 128 * (j + 1)], rhs=cf,
                        start=True, stop=True,
                    )
                    sl_ao = ao[:, SL * j : SL * (j + 1)]
                    nc.vector.tensor_mul(sl_ao, sl_ao, cb)
            return go

        def egroup(m):
            def go():
                rows = slice(SL * sc + 128 * m, SL * sc + 128 * (m + 1))
                xr = xr_p.tile([P, D], F32, tag="xr", name="xr")
                nc.sync.dma_start(out=xr, in_=d["xb"][rows, :])
                oh = oh_p.tile([P, D], F32, tag="oh", name="oh")
                pss = [psO.tile([P, 512], F32, tag="ps", name=f"pso{hf}") for hf in range(2)]
                for i in range(ND):
                    lhsT = ao[:, SL * i + 128 * m : SL * i + 128 * (m + 1)]
                    for half in range(2):
                        nc.tensor.matmul(
                            pss[half],
                            lhsT=lhsT,
                            rhs=wo[:, D * i + 512 * half : D * i + 512 * (half + 1)],
                            start=(i == 0),
                            stop=(i == ND - 1),
                            skip_group_check=True,
                        )
                for half in range(2):
                    cols = slice(512 * half, 512 * (half + 1))
                    nc.vector.tensor_add(oh[:, cols], pss[half], xr[:, cols])
                nc.sync.dma_start(out=d["out"][rows, :], in_=oh)
            return go

        for j0 in (0, 2, 4, 6):
            groups.append(dgroup(j0))
        for m in range(4):
            groups.append(egroup(m))
        return groups

    aos = {}
    fillq = []
    for sc in range(NSC):
        prefetch_xt(sc)
        qt = emit_A(sc, xts.pop(sc), fillq)
        fillq = fillq[2:] if len(fillq) > 2 else []
        ao = ao_p.tile([P, ND * SL], BF16, tag="ao", name=f"ao{sc}", bufs=4)
        aos[sc] = ao
        cfs, fillq = emit_B(sc, qt, ao, fillq)
        if cfs:
            fillq = fillq + de_groups(sc - 1, aos.pop(sc - 1), cfs[0]) + de_groups(
                sc, aos.pop(sc), cfs[1]
            )
    for g in fillq:
        g()


def build_program(n_cores=N_CORES):
    nc = bacc.Bacc(trn_type="TRN2", target_bir_lowering=False, debug=False, num_devices=n_cores)
    d = {
        "xb": nc.dram_tensor("xb", [S, D], F32, kind="ExternalInput").ap(),
        "xT": nc.dram_tensor("xT", [D, S], F32, kind="ExternalInput").ap(),
        "encT": nc.dram_tensor("encT", [D, E], F32, kind="ExternalInput").ap(),
        "selc": nc.dram_tensor("selc", [16, ND * 128], F32, kind="ExternalInput").ap(),
        "Wq": nc.dram_tensor("Wq", [D, D], F32, kind="ExternalInput").ap(),
        "Wk": nc.dram_tensor("Wk", [D, D], F32, kind="ExternalInput").ap(),
        "Wv": nc.dram_tensor("Wv", [D, D], F32, kind="ExternalInput").ap(),
        "Wo": nc.dram_tensor("Wo", [D, D], F32, kind="ExternalInput").ap(),
        
        "out": nc.dram_tensor("out", [S, D], F32, kind="ExternalOutput").ap(),
    }
    with TileContext(nc, trace_sim=False) as tc, ExitStack() as ctx:
        build_body(ctx, tc, d)
    nc.compile()
    return nc


def _selc_host():
    selc = np.zeros((16, ND * 128), np.float32)
    for j in range(ND):
        selc[15 - 2 * j, 128 * j : 128 * j + 64] = 1.0
        selc[14 - 2 * j, 128 * j + 64 : 128 * j + 128] = 1.0
    return selc


def make_in_maps(hidden_states, encoder_hidden_states, Wq, Wk, Wv, Wo, bo, n_cores=N_CORES):
    common = {
        "selc": _selc_host(),
        "Wq": np.ascontiguousarray(Wq, dtype=np.float32),
        "Wk": np.ascontiguousarray(Wk, dtype=np.float32),
        "Wv": np.ascontiguousarray(Wv, dtype=np.float32),
        "Wo": np.ascontiguousarray(Wo, dtype=np.float32),
    }
    return [
        {
            "xb": np.asarray(hidden_states[i], dtype=np.float32) + np.asarray(bo, dtype=np.float32).reshape(1, D),
            "xT": np.ascontiguousarray(np.asarray(hidden_states[i], dtype=np.float32).T),
            "encT": np.ascontiguousarray(np.asarray(encoder_hidden_states[i], dtype=np.float32).T),
            **common,
        }
        for i in range(n_cores)
    ]


def assemble(results, n_cores=N_CORES):
    return np.ascontiguousarray(
        np.stack([results[i]["out"] for i in range(n_cores)], axis=0), dtype=np.float32
    )


_NC = None


def kernel(hidden_states, encoder_hidden_states, Wq, Wk, Wv, Wo, bo):
    global _NC
    if _NC is None:
        _NC = build_program()
    in_maps = make_in_maps(hidden_states, encoder_hidden_states, Wq, Wk, Wv, Wo, bo)
    res = run_bass_kernel_spmd(_NC, in_maps, list(range(N_CORES))).results
    return assemble(res)


if __name__ == "__main__":
    build_program()
    print("compile OK")


# revision 30
# speedup vs baseline: 1.2050x; 1.2050x over previous
"""Trainium2 Bass kernel for DecomposingAttnProcessor (pooled component softmax
cross-attention), sharded over 8 NeuronCores along the batch-component axis.

Math (per batch-component bc = c*B + b):
    q = x @ Wq ; k = enc @ Wk ; v = enc @ Wv           (per-head, dh = 64)
    scores = (q k^T) * dh^-0.5                          [H, S, E]
    pooled = mean_E scores ; wp = softmax_c(pooled)     (couples components)
    w = softmax_E(scores) * wp
    out = (w v) @ Wo + bo + x

V8 design: core i owns bc = i (full S = 4096).  The only cross-component
coupling is sum_c exp(pooled) -- a [16, 512] f32 AllReduce per 512-row
s-chunk across the 4 cores sharing the same b (groups {0,2,4,6} / {1,3,5,7}),
issued right after the pooled matmuls so it completes behind the chunk's
~40us attention head loop.  Everything else is local.

The host supplies x and enc pre-transposed (xT, encT) plus the selector
constant, so the kernel contains NO PE transposes -- which lets the walrus
LDWEIGHTS optimization compile, cutting per-matmul stationary-load overhead.

  - dh^-0.5 folded into kT at the encoder stage.
  - scoresT[e, s] per head; AV head-pairs share a PSUM bank at bases 0/64 ->
    one [128, 512] eviction per pair.
  - softmax denominators emitted as a stacked [16, 512] PSUM block (own
    bank, base 0) via zero-padded ones-column matmuls (lhsT = Z[:, h:16],
    col 15 ones): den of head h lands on partition 15-h.
  - pooled rows stack at base 0 of a transient bank; exp+AllReduce up front.
  - coef = exp(pooled/E)/allsum/den on [16, 512] tiles; broadcast across
    partitions via a PE selector matmul into PSUM; DVE multiplies ao in
    place with in2 = PSUM (no DRAM bounce).
  - D/E (coef-apply + O-projection) of chunk sc-1 interleave into chunk sc's
    A/B as stall fillers.
"""

import os
import sys
from contextlib import ExitStack

sys.path.insert(0, "/opt/trn_rl_repo")

import numpy as np

import concourse.bass as bass  # noqa: E402
import concourse.bass_utils as _bass_utils  # noqa: E402
from concourse import bacc, mybir  # noqa: E402
from concourse.bass_utils import run_bass_kernel_spmd  # noqa: E402
from concourse.tile import TileContext  # noqa: E402

NO_CC = os.environ.get("NO_CC", "0") == "1"
LDW_OPT = os.environ.get("LDW_OPT", "0") == "1"

_orig_run_command = _bass_utils.run_command


def _run_command_ldwopt(argv, **kwargs):
    argv = ["--enable-ldw-opt=true" if a == "--enable-ldw-opt=false" else a for a in argv]
    return _orig_run_command(argv, **kwargs)


if LDW_OPT:
    _bass_utils.run_command = _run_command_ldwopt

# Problem dims (hardcoded per spec)
BC, S, D, E, H, C = 8, 4096, 1024, 160, 16, 4
B = BC // C  # 2
DH = D // H  # 64
SCALE = DH**-0.5  # 0.125
N_CORES = 8
E0, E1 = 128, E - 128  # encoder-token chunks (128 + 32)
ND = D // 128  # 8 chunks of the hidden dim
SL = 512  # s-chunk rows per iteration
NSC = S // SL  # 8 chunks
REPLICA_GROUPS = [[0, 2, 4, 6], [1, 3, 5, 7]]  # cores sharing the same b

F32 = mybir.dt.float32
BF16 = mybir.dt.bfloat16
EXP = mybir.ActivationFunctionType.Exp
COPY = mybir.ActivationFunctionType.Copy


def build_body(ctx, tc, d):
    nc = tc.nc
    ctx.enter_context(
        nc.allow_low_precision(reason="bf16 stats are within the 2e-2 rel-err budget")
    )
    P = 128

    pools = {}

    def pool(name, bufs, space="SBUF"):
        if name not in pools:
            pools[name] = ctx.enter_context(tc.tile_pool(name=name, bufs=bufs, space=space))
        return pools[name]

    const = pool("const", 1)
    wres = pool("wres", 1)    # Wq / Wo resident bf16
    kv_p = pool("kv", 1)      # kt / v0 / v1 / ksb / enct
    xt_p = pool("xt", 2)
    qt_p = pool("qt", 2)
    wa_p = pool("wa", 2)
    wb_p = pool("wb", 2)
    ao_p = pool("ao", 3)      # also hosts Wk/Wv during the encoder phase
    st_p = pool("st", 2)
    wstage = pool("wstage", 2)
    xr_p = pool("xr", 2)
    oh_p = pool("oh", 2)
    dram = pool("dram", 1, space="DRAM")

    # PSUM: 8 banks
    psA = pool("psA", 2, space="PSUM")    # E0 scores / kT-proj
    psEAV = pool("psEAV", 3, space="PSUM")  # E1 pairs + AV pairs / v-proj
    psO = pool("psO", 2, space="PSUM")    # Q-proj / pooled / cb / O-proj
    psDN = pool("psDN", 1, space="PSUM")  # denominator stack rows 0:16

    # ---- constants ----
    # Z: ones at col 15 only; lhsT = Z[rows, h:16] puts a ones-column at out
    # partition 15-h with zeros accumulated above it (den stack).
    zden = const.tile([P, 16], BF16, tag="zden")
    nc.vector.memset(zden, 0.0)
    nc.vector.memset(zden[:, 15:16], 1.0)
    # sel[j] = selc[:, 128j:128j+128]: cb[p, s] = coef[15 - (2j + (p>=64)), s]
    selc = const.tile([16, ND * P], BF16, tag="selc")
    nc.gpsimd.dma_start(out=selc, in_=d["selc"])

    # ---- encoder inputs first (PE starts on them), then weights ----
    enct = kv_p.tile([P, ND * E], BF16, tag="enct")
    nc.gpsimd.dma_start(
        out=enct, in_=d["encT"].rearrange("(n p) e -> p n e", p=P)
    )
    wq = wres.tile([P, ND * D], BF16, tag="wq")
    wo = wres.tile([P, ND * D], BF16, tag="wo")
    wk_lo = ao_p.tile([P, ND * SL], BF16, tag="ao", name="wk_lo", bufs=4)
    wk_hi = ao_p.tile([P, ND * SL], BF16, tag="ao", name="wk_hi", bufs=4)
    wv_lo = ao_p.tile([P, ND * SL], BF16, tag="ao", name="wv_lo", bufs=4)
    wv_hi = ao_p.tile([P, ND * SL], BF16, tag="ao", name="wv_hi", bufs=4)
    # f32 staged via the HWDGE (sync) queue + ACT cast: the gpsimd software
    # cast queue would serialize ~24MB in front of everything else.
    for nm, dsts in (("Wk", (wk_lo, wk_hi)), ("Wv", (wv_lo, wv_hi)),
                     ("Wq", (wq[:, 0 : 4 * D], wq[:, 4 * D : 8 * D])),
                     ("Wo", (wo[:, 0 : 4 * D], wo[:, 4 * D : 8 * D]))):
        for hf in range(2):
            stg = wstage.tile([P, 4 * D], F32, tag="wstg", name=f"wstg_{nm}{hf}")
            nc.sync.dma_start(
                out=stg, in_=d[nm].rearrange("(n p) d -> p n d", p=P)[:, 4 * hf : 4 * hf + 4, :]
            )
            nc.scalar.activation(dsts[hf], stg, COPY)

    def wslice(lo, hi, i, c0, c1):
        t = lo if i < 4 else hi
        return t[:, D * (i % 4) + c0 : D * (i % 4) + c1]

    # ---- encoder phase (this core's bc only) ----
    kt = kv_p.tile([P, ND * E], BF16, tag="kt")
    ksb = {}
    for j in range(ND):
        ps = psA.tile([P, 512], F32, tag="ps", name="psk")
        for i in range(ND):
            nc.tensor.matmul(
                ps[:, 0:E],
                lhsT=wslice(wk_lo, wk_hi, i, 128 * j, 128 * (j + 1)),
                rhs=enct[:, E * i : E * (i + 1)],
                start=(i == 0),
                stop=(i == ND - 1),
            )
        ksl = kt[:, E * j : E * (j + 1)]
        nc.scalar.activation(ksl, ps[:, 0:E], COPY, scale=SCALE)
        kb = kv_p.tile([P, 16], BF16, tag=f"ksb{j}", name=f"ksb{j}")
        nc.gpsimd.memset(kb, 0.0)
        # head 2j ksum -> col 15-2j (rows 0:64); head 2j+1 -> col 14-2j
        nc.vector.tensor_reduce(
            kb[0:64, 15 - 2 * j : 16 - 2 * j], ksl[0:64, :],
            axis=mybir.AxisListType.X, op=mybir.AluOpType.add,
        )
        nc.vector.tensor_reduce(
            kb[64:128, 14 - 2 * j : 15 - 2 * j], ksl[64:128, :],
            axis=mybir.AxisListType.X, op=mybir.AluOpType.add,
        )
        ksb[j] = kb
    v0 = kv_p.tile([P, D], BF16, tag="v0")
    v1 = kv_p.tile([P, D], BF16, tag="v1")
    for half in range(2):
        cols = slice(512 * half, 512 * (half + 1))
        ps0 = psEAV.tile([P, 512], F32, tag="ps", name="psv0")
        ps1 = psEAV.tile([P, 512], F32, tag="ps", name="psv1")
        for i in range(ND):
            nc.tensor.matmul(
                ps0, lhsT=enct[:, E * i : E * i + E0],
                rhs=wslice(wv_lo, wv_hi, i, 512 * half, 512 * (half + 1)),
                start=(i == 0), stop=(i == ND - 1),
            )
        for i in range(ND):
            nc.tensor.matmul(
                ps1[0:E1, :], lhsT=enct[:, E * i + E0 : E * i + E],
                rhs=wslice(wv_lo, wv_hi, i, 512 * half, 512 * (half + 1)),
                start=(i == 0), stop=(i == ND - 1),
            )
        nc.scalar.activation(v0[:, cols], ps0, COPY)
        # replicate v1 rows at partition bases 0 and 64
        nc.scalar.activation(v1[0:E1, cols], ps1[0:E1, :], COPY)
        nc.vector.tensor_copy(v1[64 : 64 + E1, cols], ps1[0:E1, :])

    # ---- main loop over s-chunks ----
    xts = {}
    state = {}

    def prefetch_xt(sc):
        if sc in xts:
            return
        xt = xt_p.tile([P, ND * SL], BF16, tag="xt", name=f"xt{sc}")
        nc.gpsimd.dma_start(
            out=xt,
            in_=d["xT"].rearrange("(n p) s -> p n s", p=P)[:, :, SL * sc : SL * (sc + 1)],
        )
        xts[sc] = xt

    def emit_A(sc, xt, filler=()):
        """Q-projection for one (prefetched, pre-transposed) s-chunk."""
        filler = list(filler)
        qt = qt_p.tile([P, ND * SL], BF16, tag="qt")
        for j in range(ND):
            if j in (2, 5) and filler:
                filler.pop(0)()
            ps = psO.tile([P, 512], F32, tag="ps", name="psq")
            for i in range(ND):
                nc.tensor.matmul(
                    ps,
                    lhsT=wq[:, D * i + 128 * j : D * i + 128 * (j + 1)],
                    rhs=xt[:, SL * i : SL * (i + 1)],
                    start=(i == 0),
                    stop=(i == ND - 1),
                )
            nc.scalar.activation(qt[:, SL * j : SL * (j + 1)], ps, COPY)
        for g in filler:
            g()
        return qt

    def emit_B(sc, qt, ao, filler):
        """Scores + exp + AV + pooled + den + coef for one chunk.

        Software-pipelined: scores(j+1) is emitted before av_den(j) so the
        exp of pair j completes behind pair j+1's score matmuls.  `filler`
        callables (prev chunk's D/E) plug the remaining gaps.
        """
        if sc + 1 < NSC:
            prefetch_xt(sc + 1)
        pl = psO.tile([P, 512], F32, tag="ps", name=f"pl{sc}")
        for j in range(ND):
            nc.tensor.matmul(
                pl[0:16, :],
                lhsT=ksb[j],
                rhs=qt[:, SL * j : SL * (j + 1)],
                start=(j == 0),
                stop=(j == ND - 1),
                skip_group_check=True,
            )
        # exp(pooled/E) + AllReduce issued up front: the collective completes
        # behind the head loop, so coef consumers never wait on it.
        ep = st_p.tile([16, SL], F32, tag="ep", name=f"ep{sc}")
        nc.scalar.activation(ep, pl[0:16, :], EXP, scale=1.0 / E)
        ep_d = dram.tile([16, SL], F32, tag="ep_d", name=f"ep_d{sc}", bufs=2)
        es_d = dram.tile([16, SL], F32, tag="es_d", name=f"es_d{sc}", bufs=2)
        nc.sync.dma_start(out=ep_d, in_=ep)
        if NO_CC:
            nc.sync.dma_start(out=es_d, in_=ep_d)
        else:
            nc.gpsimd.collective_compute(
                "AllReduce",
                mybir.AluOpType.add,
                replica_groups=REPLICA_GROUPS,
                ins=[ep_d[:, :]],
                outs=[es_d[:, :]],
            )
        es = st_p.tile([16, SL], F32, tag="es", name=f"es{sc}")
        nc.sync.dma_start(out=es, in_=es_d)
        rs = st_p.tile([16, SL], BF16, tag="rs", name=f"rs{sc}")
        nc.vector.reciprocal(rs, es)

        dn = psDN.tile([P, 512], F32, tag="ps", name=f"dn{sc}")
        fi = 0

        def scores(j):
            psb = psEAV.tile([P, 512], F32, tag="ps", name="psb")
            was = []
            for hp in range(2):
                hr = 64 * hp
                qsl = qt[hr : hr + 64, SL * j : SL * (j + 1)]
                ps_a = psA.tile([P, 512], F32, tag="ps", name="ps_a")
                nc.tensor.matmul(
                    ps_a, lhsT=kt[hr : hr + 64, E * j : E * j + E0], rhs=qsl,
                    start=True, stop=True,
                )
                nc.tensor.matmul(
                    psb[64 * hp : 64 * hp + E1, :],
                    lhsT=kt[hr : hr + 64, E * j + E0 : E * j + E],
                    rhs=qsl,
                    start=True, stop=True, skip_group_check=True,
                )
                wa = wa_p.tile([P, SL], BF16, tag=f"wa{hp}", name=f"wa{hp}")
                nc.scalar.activation(wa, ps_a, EXP)
                was.append(wa)
            wb = wb_p.tile([P, SL], BF16, tag="wb", name="wb")
            nc.scalar.activation(wb[0 : 64 + E1, :], psb[0 : 64 + E1, :], EXP)
            return was, wb

        def av_den(j, was, wb):
            ps_av = psEAV.tile([P, 512], F32, tag="ps", name="ps_av")
            for hp in range(2):
                h = 2 * j + hp
                hr = 64 * hp
                wa = was[hp]
                wbs = wb[64 * hp : 64 * hp + E1, :]
                vsl = slice(64 * h, 64 * (h + 1))
                nc.tensor.matmul(
                    ps_av[hr : hr + 64, :], lhsT=v0[:, vsl], rhs=wa,
                    start=True, stop=False, skip_group_check=True,
                )
                nc.tensor.matmul(
                    ps_av[hr : hr + 64, :],
                    lhsT=v1[64 * hp : 64 * hp + E1, vsl],
                    rhs=wbs,
                    start=False, stop=True, skip_group_check=True,
                )
                # denominator stack: den_h -> partition 15-h (base 0)
                nc.tensor.matmul(
                    dn[0 : 16 - h, :],
                    lhsT=zden[:, h:16], rhs=wa,
                    start=(h == 0), stop=False,
                    skip_group_check=True,
                )
                nc.tensor.matmul(
                    dn[0 : 16 - h, :],
                    lhsT=zden[64 * hp : 64 * hp + E1, h:16],
                    rhs=wbs,
                    start=False, stop=(h == H - 1),
                    skip_group_check=True,
                )
            nc.vector.tensor_copy(ao[:, SL * j : SL * (j + 1)], ps_av)

        prev = None
        for j in range(ND):
            cur = scores(j)
            if j == 1 and fi < len(filler):
                filler[fi]()
                fi += 1
            if prev is not None:
                av_den(j - 1, *prev)
            prev = cur
        if fi < len(filler):
            filler[fi]()
            fi += 1
        av_den(ND - 1, *prev)

        # den copied out fast so the dn bank frees before the slow DVE
        # reciprocal runs.
        dencp = st_p.tile([16, SL], F32, tag="dencp", name=f"dencp{sc}")
        nc.scalar.activation(dencp, dn[0:16, :], COPY)
        rd = st_p.tile([16, SL], BF16, tag="rd", name=f"rd{sc}")
        nc.vector.reciprocal(rd, dencp)
        cf = st_p.tile([16, SL], BF16, tag="cf", name=f"cf{sc}")
        nc.vector.tensor_mul(rd, rd, rs)
        nc.vector.tensor_mul(cf, ep, rd)
        return cf, filler[fi:]

    def de_groups(sc, ao, cf):
        """Previous chunk's D (coef apply) + E (O-proj/store) as 8 groups."""
        groups = []

        def dgroup(j0):
            def go():
                for j in (j0, j0 + 1):
                    cb = psO.tile([P, 512], F32, tag="ps", name="cb")
                    nc.tensor.matmul(
                        cb, lhsT=selc[:, 128 * j : 128 * (j + 1)], rhs=cf,
                        start=True, stop=True,
                    )
                    sl_ao = ao[:, SL * j : SL * (j + 1)]
                    nc.vector.tensor_mul(sl_ao, sl_ao, cb)
            return go

        def egroup(m):
            def go():
                rows = slice(SL * sc + 128 * m, SL * sc + 128 * (m + 1))
                xr = xr_p.tile([P, D], F32, tag="xr", name="xr")
                nc.sync.dma_start(out=xr, in_=d["xb"][rows, :])
                oh = oh_p.tile([P, D], F32, tag="oh", name="oh")
                pss = [psO.tile([P, 512], F32, tag="ps", name=f"pso{hf}") for hf in range(2)]
                for i in range(ND):
                    lhsT = ao[:, SL * i + 128 * m : SL * i + 128 * (m + 1)]
                    for half in range(2):
                        nc.tensor.matmul(
                            pss[half],
                            lhsT=lhsT,
                            rhs=wo[:, D * i + 512 * half : D * i + 512 * (half + 1)],
                            start=(i == 0),
                            stop=(i == ND - 1),
                            skip_group_check=True,
                        )
                for half in range(2):
                    cols = slice(512 * half, 512 * (half + 1))
                    nc.vector.tensor_add(oh[:, cols], pss[half], xr[:, cols])
                nc.sync.dma_start(out=d["out"][rows, :], in_=oh)
            return go

        for j0 in (0, 2, 4, 6):
            groups.append(dgroup(j0))
        for m in range(4):
            groups.append(egroup(m))
        return groups

    pend = None
    leftover = []
    for sc in range(NSC):
        prefetch_xt(sc)
        qt = emit_A(sc, xts.pop(sc), leftover)
        ao = ao_p.tile([P, ND * SL], BF16, tag="ao", name=f"ao{sc}", bufs=4)
        filler = de_groups(*pend) if pend is not None else []
        cf, leftover = emit_B(sc, qt, ao, filler)
        pend = (sc, ao, cf)
    for g in leftover:
        g()
    for g in de_groups(*pend):
        g()


def build_program(n_cores=N_CORES):
    nc = bacc.Bacc(trn_type="TRN2", target_bir_lowering=False, debug=False, num_devices=n_cores)
    d = {
        "xb": nc.dram_tensor("xb", [S, D], F32, kind="ExternalInput").ap(),
        "xT": nc.dram_tensor("xT", [D, S], F32, kind="ExternalInput").ap(),
        "encT": nc.dram_tensor("encT", [D, E], F32, kind="ExternalInput").ap(),
        "selc": nc.dram_tensor("selc", [16, ND * 128], F32, kind="ExternalInput").ap(),
        "Wq": nc.dram_tensor("Wq", [D, D], F32, kind="ExternalInput").ap(),
        "Wk": nc.dram_tensor("Wk", [D, D], F32, kind="ExternalInput").ap(),
        "Wv": nc.dram_tensor("Wv", [D, D], F32, kind="ExternalInput").ap(),
        "Wo": nc.dram_tensor("Wo", [D, D], F32, kind="ExternalInput").ap(),
        
        "out": nc.dram_tensor("out", [S, D], F32, kind="ExternalOutput").ap(),
    }
    with TileContext(nc, trace_sim=False) as tc, ExitStack() as ctx:
        build_body(ctx, tc, d)
    nc.compile()
    return nc


def _selc_host():
    selc = np.zeros((16, ND * 128), np.float32)
    for j in range(ND):
        selc[15 - 2 * j, 128 * j : 128 * j + 64] = 1.0
        selc[14 - 2 * j, 128 * j + 64 : 128 * j + 128] = 1.0
    return selc


def make_in_maps(hidden_states, encoder_hidden_states, Wq, Wk, Wv, Wo, bo, n_cores=N_CORES):
    common = {
        "selc": _selc_host(),
        "Wq": np.ascontiguousarray(Wq, dtype=np.float32),
        "Wk": np.ascontiguousarray(Wk, dtype=np.float32),
        "Wv": np.ascontiguousarray(Wv, dtype=np.float32),
        "Wo": np.ascontiguousarray(Wo, dtype=np.float32),
    }
    return [
        {
            "xb": np.asarray(hidden_states[i], dtype=np.float32) + np.asarray(bo, dtype=np.float32).reshape(1, D),
            "xT": np.ascontiguousarray(np.asarray(hidden_states[i], dtype=np.float32).T),
            "encT": np.ascontiguousarray(np.asarray(encoder_hidden_states[i], dtype=np.float32).T),
            **common,
        }
        for i in range(n_cores)
    ]


def assemble(results, n_cores=N_CORES):
    return np.ascontiguousarray(
        np.stack([results[i]["out"] for i in range(n_cores)], axis=0), dtype=np.float32
    )


_NC = None


def kernel(hidden_states, encoder_hidden_states, Wq, Wk, Wv, Wo, bo):
    global _NC
    if _NC is None:
        _NC = build_program()
    in_maps = make_in_maps(hidden_states, encoder_hidden_states, Wq, Wk, Wv, Wo, bo)
    res = run_bass_kernel_spmd(_NC, in_maps, list(range(N_CORES))).results
    return assemble(res)


if __name__ == "__main__":
    build_program()
    print("compile OK")


# revision 31
# speedup vs baseline: 1.3099x; 1.0871x over previous
"""Trainium2 Bass kernel for DecomposingAttnProcessor (pooled component softmax
cross-attention), sharded over 8 NeuronCores along the batch-component axis.

Math (per batch-component bc = c*B + b):
    q = x @ Wq ; k = enc @ Wk ; v = enc @ Wv           (per-head, dh = 64)
    scores = (q k^T) * dh^-0.5                          [H, S, E]
    pooled = mean_E scores ; wp = softmax_c(pooled)     (couples components)
    w = softmax_E(scores) * wp
    out = (w v) @ Wo + bo + x

V8 design: core i owns bc = i (full S = 4096).  The only cross-component
coupling is sum_c exp(pooled) -- a [16, 512] f32 AllReduce per 512-row
s-chunk across the 4 cores sharing the same b (groups {0,2,4,6} / {1,3,5,7}),
issued right after the pooled matmuls so it completes behind the chunk's
~40us attention head loop.  Everything else is local.

The host supplies x and enc pre-transposed (xT, encT) plus the selector
constant, so the kernel contains NO PE transposes -- which lets the walrus
LDWEIGHTS optimization compile, cutting per-matmul stationary-load overhead.

  - dh^-0.5 folded into kT at the encoder stage.
  - scoresT[e, s] per head; AV head-pairs share a PSUM bank at bases 0/64 ->
    one [128, 512] eviction per pair.
  - softmax denominators emitted as a stacked [16, 512] PSUM block (own
    bank, base 0) via zero-padded ones-column matmuls (lhsT = Z[:, h:16],
    col 15 ones): den of head h lands on partition 15-h.
  - pooled rows stack at base 0 of a transient bank; exp+AllReduce up front.
  - coef = exp(pooled/E)/allsum/den on [16, 512] tiles; broadcast across
    partitions via a PE selector matmul into PSUM; DVE multiplies ao in
    place with in2 = PSUM (no DRAM bounce).
  - D/E (coef-apply + O-projection) of chunk sc-1 interleave into chunk sc's
    A/B as stall fillers.
"""

import os
import sys
from contextlib import ExitStack

sys.path.insert(0, "/opt/trn_rl_repo")

import numpy as np

import concourse.bass as bass  # noqa: E402
import concourse.bass_utils as _bass_utils  # noqa: E402
from concourse import bacc, mybir  # noqa: E402
from concourse.bass_utils import run_bass_kernel_spmd  # noqa: E402
from concourse.tile import TileContext  # noqa: E402

NO_CC = os.environ.get("NO_CC", "0") == "1"
LDW_OPT = os.environ.get("LDW_OPT", "0") == "1"

_orig_run_command = _bass_utils.run_command


def _run_command_ldwopt(argv, **kwargs):
    argv = ["--enable-ldw-opt=true" if a == "--enable-ldw-opt=false" else a for a in argv]
    return _orig_run_command(argv, **kwargs)


if LDW_OPT:
    _bass_utils.run_command = _run_command_ldwopt

# Problem dims (hardcoded per spec)
BC, S, D, E, H, C = 8, 4096, 1024, 160, 16, 4
B = BC // C  # 2
DH = D // H  # 64
SCALE = DH**-0.5  # 0.125
N_CORES = 8
E0, E1 = 128, E - 128  # encoder-token chunks (128 + 32)
ND = D // 128  # 8 chunks of the hidden dim
SL = 512  # s-chunk rows per iteration
NSC = S // SL  # 8 chunks
REPLICA_GROUPS = [[0, 2, 4, 6], [1, 3, 5, 7]]  # cores sharing the same b

F32 = mybir.dt.float32
BF16 = mybir.dt.bfloat16
FP8 = mybir.dt.float8e4
EXP = mybir.ActivationFunctionType.Exp
COPY = mybir.ActivationFunctionType.Copy


def build_body(ctx, tc, d):
    nc = tc.nc
    ctx.enter_context(
        nc.allow_low_precision(reason="bf16 stats are within the 2e-2 rel-err budget")
    )
    P = 128

    pools = {}

    def pool(name, bufs, space="SBUF"):
        if name not in pools:
            pools[name] = ctx.enter_context(tc.tile_pool(name=name, bufs=bufs, space=space))
        return pools[name]

    const = pool("const", 1)
    wres = pool("wres", 1)    # Wq / Wo resident bf16
    kv_p = pool("kv", 1)      # kt / v0 / v1 / ksb / enct
    xt_p = pool("xt", 2)
    qt_p = pool("qt", 2)
    wa_p = pool("wa", 2)
    wb_p = pool("wb", 2)
    ao_p = pool("ao", 3)      # also hosts Wk/Wv during the encoder phase
    st_p = pool("st", 2)
    wstage = pool("wstage", 2)
    xr_p = pool("xr", 2)
    oh_p = pool("oh", 2)
    dram = pool("dram", 1, space="DRAM")

    # PSUM: 8 banks
    psA = pool("psA", 2, space="PSUM")    # E0 scores / kT-proj
    psEAV = pool("psEAV", 3, space="PSUM")  # E1 pairs + AV pairs / v-proj
    psO = pool("psO", 2, space="PSUM")    # Q-proj / pooled / cb / O-proj
    psDN = pool("psDN", 1, space="PSUM")  # denominator stack rows 0:16

    # ---- constants ----
    # Z: ones at col 15 only; lhsT = Z[rows, h:16] puts a ones-column at out
    # partition 15-h with zeros accumulated above it (den stack).
    zden = const.tile([P, 16], BF16, tag="zden")
    nc.vector.memset(zden, 0.0)
    nc.vector.memset(zden[:, 15:16], 1.0)
    # sel[j] = selc[:, 128j:128j+128]: cb[p, s] = coef[15 - (2j + (p>=64)), s]
    selc = const.tile([16, ND * P], BF16, tag="selc")
    nc.gpsimd.dma_start(out=selc, in_=d["selc"])

    # ---- encoder inputs first (PE starts on them), then weights ----
    enct = kv_p.tile([P, ND * E], BF16, tag="enct")
    nc.gpsimd.dma_start(
        out=enct, in_=d["encT"].rearrange("(n p) e -> p n e", p=P)
    )
    wq = wres.tile([P, ND * D], FP8, tag="wq")
    wo = wres.tile([P, ND * D], BF16, tag="wo")
    wk_lo = ao_p.tile([P, ND * SL], BF16, tag="ao", name="wk_lo", bufs=4)
    wk_hi = ao_p.tile([P, ND * SL], BF16, tag="ao", name="wk_hi", bufs=4)
    wv_lo = ao_p.tile([P, ND * SL], BF16, tag="ao", name="wv_lo", bufs=4)
    wv_hi = ao_p.tile([P, ND * SL], BF16, tag="ao", name="wv_hi", bufs=4)
    # f32 staged via the HWDGE (sync) queue + ACT cast: the gpsimd software
    # cast queue would serialize ~24MB in front of everything else.
    for nm, dsts in (("Wk", (wk_lo, wk_hi)), ("Wv", (wv_lo, wv_hi)),
                     ("Wq", (wq[:, 0 : 4 * D], wq[:, 4 * D : 8 * D])),
                     ("Wo", (wo[:, 0 : 4 * D], wo[:, 4 * D : 8 * D]))):
        for hf in range(2):
            stg = wstage.tile([P, 4 * D], F32, tag="wstg", name=f"wstg_{nm}{hf}")
            nc.sync.dma_start(
                out=stg, in_=d[nm].rearrange("(n p) d -> p n d", p=P)[:, 4 * hf : 4 * hf + 4, :]
            )
            nc.scalar.activation(dsts[hf], stg, COPY)

    def wslice(lo, hi, i, c0, c1):
        t = lo if i < 4 else hi
        return t[:, D * (i % 4) + c0 : D * (i % 4) + c1]

    # ---- encoder phase (this core's bc only) ----
    kt = kv_p.tile([P, ND * E], BF16, tag="kt")
    ksb = {}
    for j in range(ND):
        ps = psA.tile([P, 512], F32, tag="ps", name="psk")
        for i in range(ND):
            nc.tensor.matmul(
                ps[:, 0:E],
                lhsT=wslice(wk_lo, wk_hi, i, 128 * j, 128 * (j + 1)),
                rhs=enct[:, E * i : E * (i + 1)],
                start=(i == 0),
                stop=(i == ND - 1),
            )
        ksl = kt[:, E * j : E * (j + 1)]
        nc.scalar.activation(ksl, ps[:, 0:E], COPY, scale=SCALE)
        kb = kv_p.tile([P, 16], BF16, tag=f"ksb{j}", name=f"ksb{j}")
        nc.gpsimd.memset(kb, 0.0)
        # head 2j ksum -> col 15-2j (rows 0:64); head 2j+1 -> col 14-2j
        nc.vector.tensor_reduce(
            kb[0:64, 15 - 2 * j : 16 - 2 * j], ksl[0:64, :],
            axis=mybir.AxisListType.X, op=mybir.AluOpType.add,
        )
        nc.vector.tensor_reduce(
            kb[64:128, 14 - 2 * j : 15 - 2 * j], ksl[64:128, :],
            axis=mybir.AxisListType.X, op=mybir.AluOpType.add,
        )
        ksb[j] = kb
    v0 = kv_p.tile([P, D], BF16, tag="v0")
    v1 = kv_p.tile([P, D], BF16, tag="v1")
    for half in range(2):
        cols = slice(512 * half, 512 * (half + 1))
        ps0 = psEAV.tile([P, 512], F32, tag="ps", name="psv0")
        ps1 = psEAV.tile([P, 512], F32, tag="ps", name="psv1")
        for i in range(ND):
            nc.tensor.matmul(
                ps0, lhsT=enct[:, E * i : E * i + E0],
                rhs=wslice(wv_lo, wv_hi, i, 512 * half, 512 * (half + 1)),
                start=(i == 0), stop=(i == ND - 1),
            )
        for i in range(ND):
            nc.tensor.matmul(
                ps1[0:E1, :], lhsT=enct[:, E * i + E0 : E * i + E],
                rhs=wslice(wv_lo, wv_hi, i, 512 * half, 512 * (half + 1)),
                start=(i == 0), stop=(i == ND - 1),
            )
        nc.scalar.activation(v0[:, cols], ps0, COPY)
        # replicate v1 rows at partition bases 0 and 64
        nc.scalar.activation(v1[0:E1, cols], ps1[0:E1, :], COPY)
        nc.vector.tensor_copy(v1[64 : 64 + E1, cols], ps1[0:E1, :])

    # ---- main loop over s-chunks ----
    xts = {}
    state = {}

    def prefetch_xt(sc):
        if sc in xts:
            return
        xt = xt_p.tile([P, ND * SL], BF16, tag="xt", name=f"xt{sc}")
        nc.gpsimd.dma_start(
            out=xt,
            in_=d["xT"].rearrange("(n p) s -> p n s", p=P)[:, :, SL * sc : SL * (sc + 1)],
        )
        xt8 = xt_p.tile([P, ND * SL], FP8, tag="xt8", name=f"xt8_{sc}")
        nc.gpsimd.tensor_copy(xt8, xt)
        xts[sc] = xt8

    def emit_A(sc, xt, filler=()):
        """Q-projection for one (prefetched, pre-transposed) s-chunk."""
        filler = list(filler)
        qt = qt_p.tile([P, ND * SL], BF16, tag="qt")
        wq3 = wq.rearrange("p (n d) -> p n d", d=D)
        xt3 = xt.rearrange("p (n s) -> p n s", s=SL)
        for j in range(ND):
            if j in (2, 5) and filler:
                filler.pop(0)()
            ps = psO.tile([P, 512], F32, tag="ps", name="psq")
            for i2 in range(ND // 2):
                nc.tensor.matmul(
                    ps,
                    lhsT=wq3[:, 2 * i2 : 2 * i2 + 2, 128 * j : 128 * (j + 1)],
                    rhs=xt3[:, 2 * i2 : 2 * i2 + 2, :],
                    start=(i2 == 0),
                    stop=(i2 == ND // 2 - 1),
                    perf_mode=mybir.MatmulPerfMode.DoubleRow,
                )
            nc.scalar.activation(qt[:, SL * j : SL * (j + 1)], ps, COPY)
        for g in filler:
            g()
        return qt

    def emit_B(sc, qt, ao, filler):
        """Scores + exp + AV + pooled + den + coef for one chunk.

        Software-pipelined: scores(j+1) is emitted before av_den(j) so the
        exp of pair j completes behind pair j+1's score matmuls.  `filler`
        callables (prev chunk's D/E) plug the remaining gaps.
        """
        if sc + 1 < NSC:
            prefetch_xt(sc + 1)
        pl = psO.tile([P, 512], F32, tag="ps", name=f"pl{sc}")
        for j in range(ND):
            nc.tensor.matmul(
                pl[0:16, :],
                lhsT=ksb[j],
                rhs=qt[:, SL * j : SL * (j + 1)],
                start=(j == 0),
                stop=(j == ND - 1),
                skip_group_check=True,
            )
        # exp(pooled/E) + AllReduce issued up front: the collective completes
        # behind the head loop, so coef consumers never wait on it.
        ep = st_p.tile([16, SL], F32, tag="ep", name=f"ep{sc}")
        nc.scalar.activation(ep, pl[0:16, :], EXP, scale=1.0 / E)
        ep_d = dram.tile([16, SL], F32, tag="ep_d", name=f"ep_d{sc}", bufs=2)
        es_d = dram.tile([16, SL], F32, tag="es_d", name=f"es_d{sc}", bufs=2)
        nc.sync.dma_start(out=ep_d, in_=ep)
        if NO_CC:
            nc.sync.dma_start(out=es_d, in_=ep_d)
        else:
            nc.gpsimd.collective_compute(
                "AllReduce",
                mybir.AluOpType.add,
                replica_groups=REPLICA_GROUPS,
                ins=[ep_d[:, :]],
                outs=[es_d[:, :]],
            )
        es = st_p.tile([16, SL], F32, tag="es", name=f"es{sc}")
        nc.sync.dma_start(out=es, in_=es_d)
        rs = st_p.tile([16, SL], BF16, tag="rs", name=f"rs{sc}")
        nc.vector.reciprocal(rs, es)

        dn = psDN.tile([P, 512], F32, tag="ps", name=f"dn{sc}")
        fi = 0

        def scores(j):
            psb = psEAV.tile([P, 512], F32, tag="ps", name="psb")
            was = []
            for hp in range(2):
                hr = 64 * hp
                qsl = qt[hr : hr + 64, SL * j : SL * (j + 1)]
                ps_a = psA.tile([P, 512], F32, tag="ps", name="ps_a")
                nc.tensor.matmul(
                    ps_a, lhsT=kt[hr : hr + 64, E * j : E * j + E0], rhs=qsl,
                    start=True, stop=True,
                )
                nc.tensor.matmul(
                    psb[64 * hp : 64 * hp + E1, :],
                    lhsT=kt[hr : hr + 64, E * j + E0 : E * j + E],
                    rhs=qsl,
                    start=True, stop=True, skip_group_check=True,
                )
                wa = wa_p.tile([P, SL], BF16, tag=f"wa{hp}", name=f"wa{hp}")
                nc.scalar.activation(wa, ps_a, EXP)
                was.append(wa)
            wb = wb_p.tile([P, SL], BF16, tag="wb", name="wb")
            nc.scalar.activation(wb[0 : 64 + E1, :], psb[0 : 64 + E1, :], EXP)
            return was, wb

        def av_den(j, was, wb):
            ps_av = psEAV.tile([P, 512], F32, tag="ps", name="ps_av")
            for hp in range(2):
                h = 2 * j + hp
                hr = 64 * hp
                wa = was[hp]
                wbs = wb[64 * hp : 64 * hp + E1, :]
                vsl = slice(64 * h, 64 * (h + 1))
                nc.tensor.matmul(
                    ps_av[hr : hr + 64, :], lhsT=v0[:, vsl], rhs=wa,
                    start=True, stop=False, skip_group_check=True,
                )
                nc.tensor.matmul(
                    ps_av[hr : hr + 64, :],
                    lhsT=v1[64 * hp : 64 * hp + E1, vsl],
                    rhs=wbs,
                    start=False, stop=True, skip_group_check=True,
                )
                # denominator stack: den_h -> partition 15-h (base 0)
                nc.tensor.matmul(
                    dn[0 : 16 - h, :],
                    lhsT=zden[:, h:16], rhs=wa,
                    start=(h == 0), stop=False,
                    skip_group_check=True,
                )
                nc.tensor.matmul(
                    dn[0 : 16 - h, :],
                    lhsT=zden[64 * hp : 64 * hp + E1, h:16],
                    rhs=wbs,
                    start=False, stop=(h == H - 1),
                    skip_group_check=True,
                )
            nc.vector.tensor_copy(ao[:, SL * j : SL * (j + 1)], ps_av)

        prev = None
        for j in range(ND):
            cur = scores(j)
            if j == 1 and fi < len(filler):
                filler[fi]()
                fi += 1
            if prev is not None:
                av_den(j - 1, *prev)
            prev = cur
        if fi < len(filler):
            filler[fi]()
            fi += 1
        av_den(ND - 1, *prev)

        # den copied out fast so the dn bank frees before the slow DVE
        # reciprocal runs.
        dencp = st_p.tile([16, SL], F32, tag="dencp", name=f"dencp{sc}")
        nc.scalar.activation(dencp, dn[0:16, :], COPY)
        rd = st_p.tile([16, SL], BF16, tag="rd", name=f"rd{sc}")
        nc.vector.reciprocal(rd, dencp)
        cf = st_p.tile([16, SL], BF16, tag="cf", name=f"cf{sc}")
        nc.vector.tensor_mul(rd, rd, rs)
        nc.vector.tensor_mul(cf, ep, rd)
        return cf, filler[fi:]

    def de_groups(sc, ao, cf):
        """Previous chunk's D (coef apply) + E (O-proj/store) as 8 groups."""
        groups = []

        def dgroup(j0):
            def go():
                for j in (j0, j0 + 1):
                    cb = psO.tile([P, 512], F32, tag="ps", name="cb")
                    nc.tensor.matmul(
                        cb, lhsT=selc[:, 128 * j : 128 * (j + 1)], rhs=cf,
                        start=True, stop=True,
                    )
                    sl_ao = ao[:, SL * j : SL * (j + 1)]
                    nc.vector.tensor_mul(sl_ao, sl_ao, cb)
            return go

        def egroup(m):
            def go():
                rows = slice(SL * sc + 128 * m, SL * sc + 128 * (m + 1))
                xr = xr_p.tile([P, D], F32, tag="xr", name="xr")
                nc.sync.dma_start(out=xr, in_=d["xb"][rows, :])
                oh = oh_p.tile([P, D], F32, tag="oh", name="oh")
                pss = [psO.tile([P, 512], F32, tag="ps", name=f"pso{hf}") for hf in range(2)]
                for i in range(ND):
                    lhsT = ao[:, SL * i + 128 * m : SL * i + 128 * (m + 1)]
                    for half in range(2):
                        nc.tensor.matmul(
                            pss[half],
                            lhsT=lhsT,
                            rhs=wo[:, D * i + 512 * half : D * i + 512 * (half + 1)],
                            start=(i == 0),
                            stop=(i == ND - 1),
                            skip_group_check=True,
                        )
                for half in range(2):
                    cols = slice(512 * half, 512 * (half + 1))
                    nc.vector.tensor_add(oh[:, cols], pss[half], xr[:, cols])
                nc.sync.dma_start(out=d["out"][rows, :], in_=oh)
            return go

        for j0 in (0, 2, 4, 6):
            groups.append(dgroup(j0))
        for m in range(4):
            groups.append(egroup(m))
        return groups

    pend = None
    leftover = []
    for sc in range(NSC):
        prefetch_xt(sc)
        qt = emit_A(sc, xts.pop(sc), leftover)
        ao = ao_p.tile([P, ND * SL], BF16, tag="ao", name=f"ao{sc}", bufs=4)
        filler = de_groups(*pend) if pend is not None else []
        cf, leftover = emit_B(sc, qt, ao, filler)
        pend = (sc, ao, cf)
    for g in leftover:
        g()
    for g in de_groups(*pend):
        g()


def build_program(n_cores=N_CORES):
    nc = bacc.Bacc(trn_type="TRN2", target_bir_lowering=False, debug=False, num_devices=n_cores)
    d = {
        "xb": nc.dram_tensor("xb", [S, D], F32, kind="ExternalInput").ap(),
        "xT": nc.dram_tensor("xT", [D, S], F32, kind="ExternalInput").ap(),
        "encT": nc.dram_tensor("encT", [D, E], F32, kind="ExternalInput").ap(),
        "selc": nc.dram_tensor("selc", [16, ND * 128], F32, kind="ExternalInput").ap(),
        "Wq": nc.dram_tensor("Wq", [D, D], F32, kind="ExternalInput").ap(),
        "Wk": nc.dram_tensor("Wk", [D, D], F32, kind="ExternalInput").ap(),
        "Wv": nc.dram_tensor("Wv", [D, D], F32, kind="ExternalInput").ap(),
        "Wo": nc.dram_tensor("Wo", [D, D], F32, kind="ExternalInput").ap(),
        
        "out": nc.dram_tensor("out", [S, D], F32, kind="ExternalOutput").ap(),
    }
    with TileContext(nc, trace_sim=False) as tc, ExitStack() as ctx:
        build_body(ctx, tc, d)
    nc.compile()
    return nc


def _selc_host():
    selc = np.zeros((16, ND * 128), np.float32)
    for j in range(ND):
        selc[15 - 2 * j, 128 * j : 128 * j + 64] = 1.0
        selc[14 - 2 * j, 128 * j + 64 : 128 * j + 128] = 1.0
    return selc


def make_in_maps(hidden_states, encoder_hidden_states, Wq, Wk, Wv, Wo, bo, n_cores=N_CORES):
    common = {
        "selc": _selc_host(),
        "Wq": np.ascontiguousarray(Wq, dtype=np.float32),
        "Wk": np.ascontiguousarray(Wk, dtype=np.float32),
        "Wv": np.ascontiguousarray(Wv, dtype=np.float32),
        "Wo": np.ascontiguousarray(Wo, dtype=np.float32),
    }
    return [
        {
            "xb": np.asarray(hidden_states[i], dtype=np.float32) + np.asarray(bo, dtype=np.float32).reshape(1, D),
            "xT": np.ascontiguousarray(np.asarray(hidden_states[i], dtype=np.float32).T),
            "encT": np.ascontiguousarray(np.asarray(encoder_hidden_states[i], dtype=np.float32).T),
            **common,
        }
        for i in range(n_cores)
    ]


def assemble(results, n_cores=N_CORES):
    return np.ascontiguousarray(
        np.stack([results[i]["out"] for i in range(n_cores)], axis=0), dtype=np.float32
    )


_NC = None


def kernel(hidden_states, encoder_hidden_states, Wq, Wk, Wv, Wo, bo):
    global _NC
    if _NC is None:
        _NC = build_program()
    in_maps = make_in_maps(hidden_states, encoder_hidden_states, Wq, Wk, Wv, Wo, bo)
    res = run_bass_kernel_spmd(_NC, in_maps, list(range(N_CORES))).results
    return assemble(res)


if __name__ == "__main__":
    build_program()
    print("compile OK")


# revision 32
# speedup vs baseline: 1.3842x; 1.0567x over previous
"""Trainium2 Bass kernel for DecomposingAttnProcessor (pooled component softmax
cross-attention), sharded over 8 NeuronCores along the batch-component axis.

Math (per batch-component bc = c*B + b):
    q = x @ Wq ; k = enc @ Wk ; v = enc @ Wv           (per-head, dh = 64)
    scores = (q k^T) * dh^-0.5                          [H, S, E]
    pooled = mean_E scores ; wp = softmax_c(pooled)     (couples components)
    w = softmax_E(scores) * wp
    out = (w v) @ Wo + bo + x

V8 design: core i owns bc = i (full S = 4096).  The only cross-component
coupling is sum_c exp(pooled) -- a [16, 512] f32 AllReduce per 512-row
s-chunk across the 4 cores sharing the same b (groups {0,2,4,6} / {1,3,5,7}),
issued right after the pooled matmuls so it completes behind the chunk's
~40us attention head loop.  Everything else is local.

The host supplies x and enc pre-transposed (xT, encT) plus the selector
constant, so the kernel contains NO PE transposes -- which lets the walrus
LDWEIGHTS optimization compile, cutting per-matmul stationary-load overhead.

  - dh^-0.5 folded into kT at the encoder stage.
  - scoresT[e, s] per head; AV head-pairs share a PSUM bank at bases 0/64 ->
    one [128, 512] eviction per pair.
  - softmax denominators emitted as a stacked [16, 512] PSUM block (own
    bank, base 0) via zero-padded ones-column matmuls (lhsT = Z[:, h:16],
    col 15 ones): den of head h lands on partition 15-h.
  - pooled rows stack at base 0 of a transient bank; exp+AllReduce up front.
  - coef = exp(pooled/E)/allsum/den on [16, 512] tiles; broadcast across
    partitions via a PE selector matmul into PSUM; DVE multiplies ao in
    place with in2 = PSUM (no DRAM bounce).
  - D/E (coef-apply + O-projection) of chunk sc-1 interleave into chunk sc's
    A/B as stall fillers.
"""

import os
import sys
from contextlib import ExitStack

sys.path.insert(0, "/opt/trn_rl_repo")

import numpy as np

import concourse.bass as bass  # noqa: E402
import concourse.bass_utils as _bass_utils  # noqa: E402
from concourse import bacc, mybir  # noqa: E402
from concourse.bass_utils import run_bass_kernel_spmd  # noqa: E402
from concourse.tile import TileContext  # noqa: E402

NO_CC = os.environ.get("NO_CC", "0") == "1"
LDW_OPT = os.environ.get("LDW_OPT", "0") == "1"

_orig_run_command = _bass_utils.run_command


def _run_command_ldwopt(argv, **kwargs):
    argv = ["--enable-ldw-opt=true" if a == "--enable-ldw-opt=false" else a for a in argv]
    return _orig_run_command(argv, **kwargs)


if LDW_OPT:
    _bass_utils.run_command = _run_command_ldwopt

# Problem dims (hardcoded per spec)
BC, S, D, E, H, C = 8, 4096, 1024, 160, 16, 4
B = BC // C  # 2
DH = D // H  # 64
SCALE = DH**-0.5  # 0.125
N_CORES = 8
E0, E1 = 128, E - 128  # encoder-token chunks (128 + 32)
ND = D // 128  # 8 chunks of the hidden dim
SL = 512  # s-chunk rows per iteration
NSC = S // SL  # 8 chunks
REPLICA_GROUPS = [[0, 2, 4, 6], [1, 3, 5, 7]]  # cores sharing the same b

F32 = mybir.dt.float32
BF16 = mybir.dt.bfloat16
FP8 = mybir.dt.float8e4
EXP = mybir.ActivationFunctionType.Exp
COPY = mybir.ActivationFunctionType.Copy


def build_body(ctx, tc, d):
    nc = tc.nc
    ctx.enter_context(
        nc.allow_low_precision(reason="bf16 stats are within the 2e-2 rel-err budget")
    )
    P = 128

    pools = {}

    def pool(name, bufs, space="SBUF"):
        if name not in pools:
            pools[name] = ctx.enter_context(tc.tile_pool(name=name, bufs=bufs, space=space))
        return pools[name]

    const = pool("const", 1)
    wres = pool("wres", 1)    # Wq / Wo resident bf16
    kv_p = pool("kv", 1)      # kt / v0 / v1 / ksb / enct
    xt_p = pool("xt", 2)
    qt_p = pool("qt", 2)
    wa_p = pool("wa", 2)
    wb_p = pool("wb", 2)
    ao_p = pool("ao", 3)      # also hosts Wk/Wv during the encoder phase
    st_p = pool("st", 2)
    ao8_p = pool("ao8", 2)
    wstage = pool("wstage", 2)
    xr_p = pool("xr", 2)
    oh_p = pool("oh", 2)
    dram = pool("dram", 1, space="DRAM")

    # PSUM: 8 banks
    psA = pool("psA", 2, space="PSUM")    # E0 scores / kT-proj
    psEAV = pool("psEAV", 3, space="PSUM")  # E1 pairs + AV pairs / v-proj
    psO = pool("psO", 2, space="PSUM")    # Q-proj / pooled / cb / O-proj
    psDN = pool("psDN", 1, space="PSUM")  # denominator stack rows 0:16

    # ---- constants ----
    # Z: ones at col 15 only; lhsT = Z[rows, h:16] puts a ones-column at out
    # partition 15-h with zeros accumulated above it (den stack).
    zden = const.tile([P, 16], BF16, tag="zden")
    nc.vector.memset(zden, 0.0)
    nc.vector.memset(zden[:, 15:16], 1.0)
    # sel[j] = selc[:, 128j:128j+128]: cb[p, s] = coef[15 - (2j + (p>=64)), s]
    selc = const.tile([16, ND * P], BF16, tag="selc")
    nc.gpsimd.dma_start(out=selc, in_=d["selc"])

    # ---- encoder inputs first (PE starts on them), then weights ----
    enct = kv_p.tile([P, ND * E], BF16, tag="enct")
    nc.gpsimd.dma_start(
        out=enct, in_=d["encT"].rearrange("(n p) e -> p n e", p=P)
    )
    wq = wres.tile([P, ND * D], FP8, tag="wq")
    wo = wres.tile([P, ND * D], FP8, tag="wo")
    wk_lo = ao_p.tile([P, ND * SL], BF16, tag="ao", name="wk_lo", bufs=4)
    wk_hi = ao_p.tile([P, ND * SL], BF16, tag="ao", name="wk_hi", bufs=4)
    wv_lo = ao_p.tile([P, ND * SL], BF16, tag="ao", name="wv_lo", bufs=4)
    wv_hi = ao_p.tile([P, ND * SL], BF16, tag="ao", name="wv_hi", bufs=4)
    # f32 staged via the HWDGE (sync) queue + ACT cast: the gpsimd software
    # cast queue would serialize ~24MB in front of everything else.
    for nm, dsts in (("Wk", (wk_lo, wk_hi)), ("Wv", (wv_lo, wv_hi)),
                     ("Wq", (wq[:, 0 : 4 * D], wq[:, 4 * D : 8 * D])),
                     ("Wo", (wo[:, 0 : 4 * D], wo[:, 4 * D : 8 * D]))):
        for hf in range(2):
            stg = wstage.tile([P, 4 * D], F32, tag="wstg", name=f"wstg_{nm}{hf}")
            nc.sync.dma_start(
                out=stg, in_=d[nm].rearrange("(n p) d -> p n d", p=P)[:, 4 * hf : 4 * hf + 4, :]
            )
            nc.scalar.activation(dsts[hf], stg, COPY)

    def wslice(lo, hi, i, c0, c1):
        t = lo if i < 4 else hi
        return t[:, D * (i % 4) + c0 : D * (i % 4) + c1]

    # ---- encoder phase (this core's bc only) ----
    kt = kv_p.tile([P, ND * E], BF16, tag="kt")
    ksb = {}
    for j in range(ND):
        ps = psA.tile([P, 512], F32, tag="ps", name="psk")
        for i in range(ND):
            nc.tensor.matmul(
                ps[:, 0:E],
                lhsT=wslice(wk_lo, wk_hi, i, 128 * j, 128 * (j + 1)),
                rhs=enct[:, E * i : E * (i + 1)],
                start=(i == 0),
                stop=(i == ND - 1),
            )
        ksl = kt[:, E * j : E * (j + 1)]
        nc.scalar.activation(ksl, ps[:, 0:E], COPY, scale=SCALE)
        kb = kv_p.tile([P, 16], BF16, tag=f"ksb{j}", name=f"ksb{j}")
        nc.gpsimd.memset(kb, 0.0)
        # head 2j ksum -> col 15-2j (rows 0:64); head 2j+1 -> col 14-2j
        nc.vector.tensor_reduce(
            kb[0:64, 15 - 2 * j : 16 - 2 * j], ksl[0:64, :],
            axis=mybir.AxisListType.X, op=mybir.AluOpType.add,
        )
        nc.vector.tensor_reduce(
            kb[64:128, 14 - 2 * j : 15 - 2 * j], ksl[64:128, :],
            axis=mybir.AxisListType.X, op=mybir.AluOpType.add,
        )
        ksb[j] = kb
    v0 = kv_p.tile([P, D], BF16, tag="v0")
    v1 = kv_p.tile([P, D], BF16, tag="v1")
    for half in range(2):
        cols = slice(512 * half, 512 * (half + 1))
        ps0 = psEAV.tile([P, 512], F32, tag="ps", name="psv0")
        ps1 = psEAV.tile([P, 512], F32, tag="ps", name="psv1")
        for i in range(ND):
            nc.tensor.matmul(
                ps0, lhsT=enct[:, E * i : E * i + E0],
                rhs=wslice(wv_lo, wv_hi, i, 512 * half, 512 * (half + 1)),
                start=(i == 0), stop=(i == ND - 1),
            )
        for i in range(ND):
            nc.tensor.matmul(
                ps1[0:E1, :], lhsT=enct[:, E * i + E0 : E * i + E],
                rhs=wslice(wv_lo, wv_hi, i, 512 * half, 512 * (half + 1)),
                start=(i == 0), stop=(i == ND - 1),
            )
        nc.scalar.activation(v0[:, cols], ps0, COPY)
        # replicate v1 rows at partition bases 0 and 64
        nc.scalar.activation(v1[0:E1, cols], ps1[0:E1, :], COPY)
        nc.vector.tensor_copy(v1[64 : 64 + E1, cols], ps1[0:E1, :])

    # ---- main loop over s-chunks ----
    xts = {}
    state = {}

    def prefetch_xt(sc):
        if sc in xts:
            return
        xt = xt_p.tile([P, ND * SL], BF16, tag="xt", name=f"xt{sc}")
        nc.gpsimd.dma_start(
            out=xt,
            in_=d["xT"].rearrange("(n p) s -> p n s", p=P)[:, :, SL * sc : SL * (sc + 1)],
        )
        xt8 = xt_p.tile([P, ND * SL], FP8, tag="xt8", name=f"xt8_{sc}")
        nc.gpsimd.tensor_copy(xt8, xt)
        xts[sc] = xt8

    def emit_A(sc, xt, filler=()):
        """Q-projection for one (prefetched, pre-transposed) s-chunk."""
        filler = list(filler)
        qt = qt_p.tile([P, ND * SL], BF16, tag="qt")
        wq3 = wq.rearrange("p (n d) -> p n d", d=D)
        xt3 = xt.rearrange("p (n s) -> p n s", s=SL)
        for j in range(ND):
            if j in (2, 5) and filler:
                filler.pop(0)()
            ps = psO.tile([P, 512], F32, tag="ps", name="psq")
            for i2 in range(ND // 2):
                nc.tensor.matmul(
                    ps,
                    lhsT=wq3[:, 2 * i2 : 2 * i2 + 2, 128 * j : 128 * (j + 1)],
                    rhs=xt3[:, 2 * i2 : 2 * i2 + 2, :],
                    start=(i2 == 0),
                    stop=(i2 == ND // 2 - 1),
                    perf_mode=mybir.MatmulPerfMode.DoubleRow,
                )
            nc.scalar.activation(qt[:, SL * j : SL * (j + 1)], ps, COPY)
        for g in filler:
            g()
        return qt

    def emit_B(sc, qt, ao, filler):
        """Scores + exp + AV + pooled + den + coef for one chunk.

        Software-pipelined: scores(j+1) is emitted before av_den(j) so the
        exp of pair j completes behind pair j+1's score matmuls.  `filler`
        callables (prev chunk's D/E) plug the remaining gaps.
        """
        if sc + 1 < NSC:
            prefetch_xt(sc + 1)
        pl = psO.tile([P, 512], F32, tag="ps", name=f"pl{sc}")
        for j in range(ND):
            nc.tensor.matmul(
                pl[0:16, :],
                lhsT=ksb[j],
                rhs=qt[:, SL * j : SL * (j + 1)],
                start=(j == 0),
                stop=(j == ND - 1),
                skip_group_check=True,
            )
        # exp(pooled/E) + AllReduce issued up front: the collective completes
        # behind the head loop, so coef consumers never wait on it.
        ep = st_p.tile([16, SL], F32, tag="ep", name=f"ep{sc}")
        nc.scalar.activation(ep, pl[0:16, :], EXP, scale=1.0 / E)
        ep_d = dram.tile([16, SL], F32, tag="ep_d", name=f"ep_d{sc}", bufs=2)
        es_d = dram.tile([16, SL], F32, tag="es_d", name=f"es_d{sc}", bufs=2)
        nc.sync.dma_start(out=ep_d, in_=ep)
        if NO_CC:
            nc.sync.dma_start(out=es_d, in_=ep_d)
        else:
            nc.gpsimd.collective_compute(
                "AllReduce",
                mybir.AluOpType.add,
                replica_groups=REPLICA_GROUPS,
                ins=[ep_d[:, :]],
                outs=[es_d[:, :]],
            )
        es = st_p.tile([16, SL], F32, tag="es", name=f"es{sc}")
        nc.sync.dma_start(out=es, in_=es_d)
        rs = st_p.tile([16, SL], BF16, tag="rs", name=f"rs{sc}")
        nc.vector.reciprocal(rs, es)

        dn = psDN.tile([P, 512], F32, tag="ps", name=f"dn{sc}")
        fi = 0

        def scores(j):
            psb = psEAV.tile([P, 512], F32, tag="ps", name="psb")
            was = []
            for hp in range(2):
                hr = 64 * hp
                qsl = qt[hr : hr + 64, SL * j : SL * (j + 1)]
                ps_a = psA.tile([P, 512], F32, tag="ps", name="ps_a")
                nc.tensor.matmul(
                    ps_a, lhsT=kt[hr : hr + 64, E * j : E * j + E0], rhs=qsl,
                    start=True, stop=True,
                )
                nc.tensor.matmul(
                    psb[64 * hp : 64 * hp + E1, :],
                    lhsT=kt[hr : hr + 64, E * j + E0 : E * j + E],
                    rhs=qsl,
                    start=True, stop=True, skip_group_check=True,
                )
                wa = wa_p.tile([P, SL], BF16, tag=f"wa{hp}", name=f"wa{hp}")
                nc.scalar.activation(wa, ps_a, EXP)
                was.append(wa)
            wb = wb_p.tile([P, SL], BF16, tag="wb", name="wb")
            nc.scalar.activation(wb[0 : 64 + E1, :], psb[0 : 64 + E1, :], EXP)
            return was, wb

        def av_den(j, was, wb):
            ps_av = psEAV.tile([P, 512], F32, tag="ps", name="ps_av")
            for hp in range(2):
                h = 2 * j + hp
                hr = 64 * hp
                wa = was[hp]
                wbs = wb[64 * hp : 64 * hp + E1, :]
                vsl = slice(64 * h, 64 * (h + 1))
                nc.tensor.matmul(
                    ps_av[hr : hr + 64, :], lhsT=v0[:, vsl], rhs=wa,
                    start=True, stop=False, skip_group_check=True,
                )
                nc.tensor.matmul(
                    ps_av[hr : hr + 64, :],
                    lhsT=v1[64 * hp : 64 * hp + E1, vsl],
                    rhs=wbs,
                    start=False, stop=True, skip_group_check=True,
                )
                # denominator stack: den_h -> partition 15-h (base 0)
                nc.tensor.matmul(
                    dn[0 : 16 - h, :],
                    lhsT=zden[:, h:16], rhs=wa,
                    start=(h == 0), stop=False,
                    skip_group_check=True,
                )
                nc.tensor.matmul(
                    dn[0 : 16 - h, :],
                    lhsT=zden[64 * hp : 64 * hp + E1, h:16],
                    rhs=wbs,
                    start=False, stop=(h == H - 1),
                    skip_group_check=True,
                )
            nc.vector.tensor_copy(ao[:, SL * j : SL * (j + 1)], ps_av)

        prev = None
        for j in range(ND):
            cur = scores(j)
            if j == 1 and fi < len(filler):
                filler[fi]()
                fi += 1
            if prev is not None:
                av_den(j - 1, *prev)
            prev = cur
        if fi < len(filler):
            filler[fi]()
            fi += 1
        av_den(ND - 1, *prev)

        # den copied out fast so the dn bank frees before the slow DVE
        # reciprocal runs.
        dencp = st_p.tile([16, SL], F32, tag="dencp", name=f"dencp{sc}")
        nc.scalar.activation(dencp, dn[0:16, :], COPY)
        rd = st_p.tile([16, SL], BF16, tag="rd", name=f"rd{sc}")
        nc.vector.reciprocal(rd, dencp)
        cf = st_p.tile([16, SL], BF16, tag="cf", name=f"cf{sc}")
        nc.vector.tensor_mul(rd, rd, rs)
        nc.vector.tensor_mul(cf, ep, rd)
        return cf, filler[fi:]

    ao8s = {}

    def de_groups(sc, ao, cf):
        """Previous chunk's D (coef apply) + E (O-proj/store) as 8 groups."""
        groups = []
        ao8 = ao8_p.tile([P, ND * SL], FP8, tag="ao8", name=f"ao8_{sc}")
        ao8s[sc] = ao8

        def dgroup(j0):
            def go():
                for j in (j0, j0 + 1):
                    cb = psO.tile([P, 512], F32, tag="ps", name="cb")
                    nc.tensor.matmul(
                        cb, lhsT=selc[:, 128 * j : 128 * (j + 1)], rhs=cf,
                        start=True, stop=True,
                    )
                    nc.vector.tensor_mul(
                        ao8[:, SL * j : SL * (j + 1)], ao[:, SL * j : SL * (j + 1)], cb
                    )
            return go

        def egroup(m):
            def go():
                rows = slice(SL * sc + 128 * m, SL * sc + 128 * (m + 1))
                xr = xr_p.tile([P, D], F32, tag="xr", name="xr")
                nc.sync.dma_start(out=xr, in_=d["xb"][rows, :])
                oh = oh_p.tile([P, D], F32, tag="oh", name="oh")
                pss = [psO.tile([P, 512], F32, tag="ps", name=f"pso{hf}") for hf in range(2)]
                ao3 = ao8.rearrange("p (n s) -> p n s", s=SL)
                wo3 = wo.rearrange("p (n d) -> p n d", d=D)
                for i2 in range(ND // 2):
                    lhsT = ao3[:, 2 * i2 : 2 * i2 + 2, 128 * m : 128 * (m + 1)]
                    for half in range(2):
                        nc.tensor.matmul(
                            pss[half],
                            lhsT=lhsT,
                            rhs=wo3[:, 2 * i2 : 2 * i2 + 2, 512 * half : 512 * (half + 1)],
                            start=(i2 == 0),
                            stop=(i2 == ND // 2 - 1),
                            skip_group_check=True,
                            perf_mode=mybir.MatmulPerfMode.DoubleRow,
                        )
                for half in range(2):
                    cols = slice(512 * half, 512 * (half + 1))
                    nc.vector.tensor_add(oh[:, cols], pss[half], xr[:, cols])
                nc.sync.dma_start(out=d["out"][rows, :], in_=oh)
            return go

        for j0 in (0, 2, 4, 6):
            groups.append(dgroup(j0))
        for m in range(4):
            groups.append(egroup(m))
        return groups

    pend = None
    leftover = []
    for sc in range(NSC):
        prefetch_xt(sc)
        qt = emit_A(sc, xts.pop(sc), leftover)
        ao = ao_p.tile([P, ND * SL], BF16, tag="ao", name=f"ao{sc}", bufs=4)
        filler = de_groups(*pend) if pend is not None else []
        cf, leftover = emit_B(sc, qt, ao, filler)
        pend = (sc, ao, cf)
    for g in leftover:
        g()
    for g in de_groups(*pend):
        g()


def build_program(n_cores=N_CORES):
    nc = bacc.Bacc(trn_type="TRN2", target_bir_lowering=False, debug=False, num_devices=n_cores)
    d = {
        "xb": nc.dram_tensor("xb", [S, D], F32, kind="ExternalInput").ap(),
        "xT": nc.dram_tensor("xT", [D, S], F32, kind="ExternalInput").ap(),
        "encT": nc.dram_tensor("encT", [D, E], F32, kind="ExternalInput").ap(),
        "selc": nc.dram_tensor("selc", [16, ND * 128], F32, kind="ExternalInput").ap(),
        "Wq": nc.dram_tensor("Wq", [D, D], F32, kind="ExternalInput").ap(),
        "Wk": nc.dram_tensor("Wk", [D, D], F32, kind="ExternalInput").ap(),
        "Wv": nc.dram_tensor("Wv", [D, D], F32, kind="ExternalInput").ap(),
        "Wo": nc.dram_tensor("Wo", [D, D], F32, kind="ExternalInput").ap(),
        
        "out": nc.dram_tensor("out", [S, D], F32, kind="ExternalOutput").ap(),
    }
    with TileContext(nc, trace_sim=False) as tc, ExitStack() as ctx:
        build_body(ctx, tc, d)
    nc.compile()
    return nc


def _selc_host():
    selc = np.zeros((16, ND * 128), np.float32)
    for j in range(ND):
        selc[15 - 2 * j, 128 * j : 128 * j + 64] = 1.0
        selc[14 - 2 * j, 128 * j + 64 : 128 * j + 128] = 1.0
    return selc


def make_in_maps(hidden_states, encoder_hidden_states, Wq, Wk, Wv, Wo, bo, n_cores=N_CORES):
    common = {
        "selc": _selc_host(),
        "Wq": np.ascontiguousarray(Wq, dtype=np.float32),
        "Wk": np.ascontiguousarray(Wk, dtype=np.float32),
        "Wv": np.ascontiguousarray(Wv, dtype=np.float32),
        "Wo": np.ascontiguousarray(Wo, dtype=np.float32),
    }
    return [
        {
            "xb": np.asarray(hidden_states[i], dtype=np.float32) + np.asarray(bo, dtype=np.float32).reshape(1, D),
            "xT": np.ascontiguousarray(np.asarray(hidden_states[i], dtype=np.float32).T),
            "encT": np.ascontiguousarray(np.asarray(encoder_hidden_states[i], dtype=np.float32).T),
            **common,
        }
        for i in range(n_cores)
    ]


def assemble(results, n_cores=N_CORES):
    return np.ascontiguousarray(
        np.stack([results[i]["out"] for i in range(n_cores)], axis=0), dtype=np.float32
    )


_NC = None


def kernel(hidden_states, encoder_hidden_states, Wq, Wk, Wv, Wo, bo):
    global _NC
    if _NC is None:
        _NC = build_program()
    in_maps = make_in_maps(hidden_states, encoder_hidden_states, Wq, Wk, Wv, Wo, bo)
    res = run_bass_kernel_spmd(_NC, in_maps, list(range(N_CORES))).results
    return assemble(res)


if __name__ == "__main__":
    build_program()
    print("compile OK")
